# revision 1
# baseline (speedup 1.0000x reference)
"""Decorrelation (ZCA-whitening) normalization kernel for Trainium2 (Bass/Tile).

Full input (64, 56, 56, 256) f32. Data-parallel over batch across 8 NeuronCores
(8 batches -> 25088 pixels per core). Per core:

  Pass 1: stream pixel-major (128px, 14, 256ch) f32 chunks from HBM, cast to
          fp16, accumulate per-half 128x128 second-moment Gram blocks on the
          PE (PSUM f32), PE-transpose every (128px,128ch) tile to channel-major
          fp16 and keep it fully SBUF-resident (12.8 MB), reduce channel sums.
  Stats:  one 132KB AllReduce of [G_a | G_b | sum_a | sum_b] across the 8
          cores; each core then runs the tiny (2 x 128x128 block-diagonal)
          Newton-Schulz iteration in f32 on-device, producing the whitening
          matrix wm (fp16) and the -mean bias.
  Pass 2: subtract mean in-place on the channel-major resident tiles (ACT,
          per-partition bias), whitening matmul lhsT=resident_slice rhs=wm
          (fp16 -> PSUM f32, output pixel-major), copy to staging, DMA out.

HBM traffic per core = 1x read + 1x write (pass 2 reads nothing from HBM).
"""

import sys

import numpy as np

for _p in ("/root/.axon_site/_ro/trn_rl_repo", "/opt/trn_rl_repo"):
    if _p not in sys.path:
        sys.path.append(_p)

# ---------------------------------------------------------------- constants
B, W, H, C = 64, 56, 56, 256
N_CORES = 8
B_LOC = B // N_CORES                # 8 batches per core
N_LOC = B_LOC * W * H               # 25088 pixels per core
N_TOT = B * W * H                   # 200704 pixels total
P = 128                             # partitions
UJ = 14                             # pixel-tiles (units) per chunk
CPX = UJ * P                        # 1792 pixels per chunk
NCHUNK = N_LOC // CPX               # 14 chunks per core
EPS = 1e-3
ITER_NUM = 5

assert NCHUNK * CPX == N_LOC

_STATE = {}


def _build_nc(variant=()):
    import concourse.bacc as bacc
    import concourse.tile as tile
    from concourse import mybir
    from contextlib import ExitStack

    f32 = mybir.dt.float32
    f16 = mybir.dt.float16
    Alu = mybir.AluOpType
    Act = mybir.ActivationFunctionType
    Axis = mybir.AxisListType

    nc = bacc.Bacc("TRN2", target_bir_lowering=False, debug=False,
                   num_devices=N_CORES)

    x = nc.dram_tensor("x", [N_LOC, C], f32, kind="ExternalInput").ap()
    y = nc.dram_tensor("y", [N_LOC, C], f32, kind="ExternalOutput").ap()
    c_id16 = nc.dram_tensor("c_id16", [P, P], f16, kind="ExternalInput").ap()
    c_eye = nc.dram_tensor("c_eye", [P, P], f32, kind="ExternalInput").ap()
    c_epseye = nc.dram_tensor("c_epseye", [P, P], f32, kind="ExternalInput").ap()
    c_mask = nc.dram_tensor("c_mask", [P, P], f32, kind="ExternalInput").ap()

    with tile.TileContext(nc) as tc, ExitStack() as octx:
        # ---------------- long-lived pools
        consts = octx.enter_context(tc.tile_pool(name="consts", bufs=1))
        resp = octx.enter_context(tc.tile_pool(name="resident", bufs=1))
        statp = octx.enter_context(tc.tile_pool(name="stats", bufs=1))

        id16 = consts.tile([P, P], f16, name="id16")
        eye = consts.tile([P, P], f32, name="eye")
        epseye = consts.tile([P, P], f32, name="epseye")
        mask = consts.tile([P, P], f32, name="mask")
        nc.gpsimd.dma_start(out=id16, in_=c_id16)
        nc.gpsimd.dma_start(out=eye, in_=c_eye)
        nc.gpsimd.dma_start(out=epseye, in_=c_epseye)
        nc.gpsimd.dma_start(out=mask, in_=c_mask)

        # stats block: [G_a | G_b | sum_a | sum_b] (128, 258) f32
        statsb = statp.tile([P, 2 * P + 2], f32, name="statsb")
        nc.vector.memset(statsb, 0.0)

        # channel-major fp16 resident tiles: one per (chunk, half)
        res = [[resp.tile([P, UJ, P], f16, name=f"res_{c}_{h}")
                for h in range(2)] for c in range(NCHUNK)]

        xv = x.rearrange("(c j p) ch -> c p j ch", p=P, j=UJ)
        yv = y.rearrange("(c j p) ch -> c p j ch", p=P, j=UJ)

        nrep = 1
        for v in variant:
            if v.startswith("rep"):
                nrep = int(v[3:])
        for _rep in range(nrep):
         # ================= PASS 1 =================
         with ExitStack() as ctx:
             loadp = ctx.enter_context(tc.tile_pool(name="loadp", bufs=2))
             castp = ctx.enter_context(tc.tile_pool(name="castp", bufs=2))
             gps = ctx.enter_context(tc.tile_pool(name="gpsum", bufs=1, space="PSUM"))
             trps = ctx.enter_context(tc.tile_pool(name="trpsum", bufs=4, space="PSUM"))

             g_ps = [gps.tile([P, P], f32, name=f"G_{h}") for h in range(2)]

             for ci in range(NCHUNK):
                 xt = loadp.tile([P, UJ, C], f32, name="xt")
                 nc.gpsimd.dma_start(out=xt, in_=xv[ci])
                 xh = castp.tile([P, UJ, C], f16, name="xh")
                 nc.vector.tensor_copy(out=xh, in_=xt)

                 # Gram accumulation (fp16 in, f32 PSUM): G_h += T_h^T @ T_h
                 for j in range(UJ):
                     first = ci == 0 and j == 0
                     last = ci == NCHUNK - 1 and j == UJ - 1
                     if "nogram" in variant:
                         continue
                     for h in range(2):
                         sl = xh[:, j, h * P:(h + 1) * P]
                         nc.tensor.matmul(g_ps[h], sl, sl, start=first,
                                          stop=last, skip_group_check=True)

                 # PE transpose each (128px,128ch) tile -> channel-major fp16
                 for h in range(2 if "notr" not in variant else 0):
                     for b0 in range(0, UJ, 4):
                         bn = min(4, UJ - b0)
                         tp = trps.tile([P, 4, P], f16, name="tp")
                         for k in range(bn):
                             nc.tensor.matmul(
                                 tp[:, k, :], xh[:, b0 + k, h * P:(h + 1) * P],
                                 id16, is_transpose=True, skip_group_check=True)
                         nc.scalar.activation(
                             out=res[ci][h][:, b0:b0 + bn, :], in_=tp[:, :bn, :],
                             func=Act.Copy)

                 # channel sums from the (already rounded) fp16 resident tiles
                 for h in range(2 if "notr" not in variant else 0):
                     csum = castp.tile([P, 1], f32, name="csum")
                     nc.vector.tensor_reduce(out=csum, in_=res[ci][h],
                                             axis=Axis.XY, op=Alu.add)
                     nc.vector.tensor_add(
                         out=statsb[:, 2 * P + h:2 * P + h + 1],
                         in0=statsb[:, 2 * P + h:2 * P + h + 1], in1=csum)

             # move Gram PSUM -> stats block
             for h in range(2 if "nogram" not in variant else 0):
                 nc.scalar.activation(out=statsb[:, h * P:(h + 1) * P],
                                      in_=g_ps[h], func=Act.Copy)

         # ================= ALL-REDUCE =================
         with ExitStack() as ctx:
             dramp = ctx.enter_context(tc.tile_pool(name="dram", bufs=1, space="DRAM"))
             cc_in = dramp.tile([P, 2 * P + 2], f32, name="cc_in")
             cc_out = dramp.tile([P, 2 * P + 2], f32, name="cc_out")
             arst = statp.tile([P, 2 * P + 2], f32, name="arst")
             if "nocc" in variant:
                 nc.vector.tensor_scalar_mul(out=arst, in0=statsb,
                                             scalar1=float(N_CORES))
             else:
                 nc.gpsimd.dma_start(out=cc_in, in_=statsb)
                 nc.gpsimd.collective_compute(
                     "AllReduce", mybir.AluOpType.add,
                     replica_groups=[list(range(N_CORES))],
                     ins=[cc_in.opt()], outs=[cc_out.opt()])
                 nc.gpsimd.dma_start(out=arst, in_=cc_out)

             # ============= Newton-Schulz (per half) =============
             nsp = ctx.enter_context(tc.tile_pool(name="nsp", bufs=6))
             nps = ctx.enter_context(tc.tile_pool(name="nspsum", bufs=4, space="PSUM"))

             wm16 = [statp.tile([P, P], f16, name=f"wm16_{h}") for h in range(2)]
             nmu = [statp.tile([P, 1], f32, name=f"nmu_{h}") for h in range(2)]

             for h in range(2):
                 arG = arst[:, h * P:(h + 1) * P]
                 s_col = arst[:, 2 * P + h:2 * P + h + 1]

                 # -mean column (bias for pass 2)
                 nc.scalar.activation(out=nmu[h], in_=s_col, func=Act.Identity,
                                      scale=-1.0 / N_TOT)

                 # mu as row 0 of a zero (128,128) tile, via PE transpose
                 colpad = nsp.tile([P, P], f32, name="colpad", tag="nsbig")
                 nc.vector.memset(colpad, 0.0)
                 nc.scalar.activation(out=colpad[:, 0:1], in_=s_col,
                                      func=Act.Identity, scale=1.0 / N_TOT)
                 rp_ps = nps.tile([P, P], f32, name="rp_ps", tag="nsps")
                 nc.tensor.matmul(rp_ps, colpad, eye, is_transpose=True,
                                  skip_group_check=True)
                 rowpad = nsp.tile([P, P], f32, name="rowpad", tag="nsbig")
                 nc.scalar.activation(out=rowpad, in_=rp_ps, func=Act.Copy)

                 # outer product mu mu^T (only row 0 of rowpad is nonzero)
                 o_ps = nps.tile([P, P], f32, name="o_ps", tag="nsps")
                 nc.tensor.matmul(o_ps, rowpad, rowpad, skip_group_check=True)
                 osc = nsp.tile([P, P], f32, name="osc", tag="nsbig")
                 nc.scalar.activation(out=osc, in_=o_ps, func=Act.Identity,
                                      scale=-(1.0 - EPS))

                 # sigma = mask * ((1-eps)/N * G - (1-eps) * mu mu^T) + eps*I
                 sig = nsp.tile([P, P], f32, name="sig", tag="sig")
                 nc.vector.scalar_tensor_tensor(
                     out=sig, in0=arG, scalar=(1.0 - EPS) / N_TOT, in1=osc,
                     op0=Alu.mult, op1=Alu.add)
                 nc.vector.tensor_mul(out=sig, in0=sig, in1=mask)
                 nc.vector.tensor_add(out=sig, in0=sig, in1=epseye)

                 # per-group trace, spread back to rows via mask matmul
                 djunk = nsp.tile([P, P], f32, name="djunk", tag="nsbig")
                 dcol = nsp.tile([P, 1], f32, name="dcol", tag="nssmall")
                 nc.vector.tensor_mul(out=djunk, in0=sig, in1=eye)
                 nc.vector.reduce_sum(out=dcol, in_=djunk, axis=Axis.X)
                 tv_ps = nps.tile([P, 1], f32, name="tv_ps", tag="nsps")
                 nc.tensor.matmul(tv_ps, mask, dcol, skip_group_check=True)
                 tvec = nsp.tile([P, 1], f32, name="tvec", tag="nssmall")
                 nc.scalar.activation(out=tvec, in_=tv_ps, func=Act.Copy)
                 rinv = nsp.tile([P, 1], f32, name="rinv", tag="nssmall")
                 nc.vector.reciprocal(out=rinv, in_=tvec)

                 sign = nsp.tile([P, P], f32, name="sign", tag="sign")
                 nc.vector.tensor_scalar_mul(out=sign, in0=sig, scalar1=rinv)

                 # P_{k+1} = 1.5 P - 0.5 P^3 sigma_n ; P_0 = I
                 ps_t = nsp.tile([P, P], f32, name=f"ps_{h}", tag="ps")
                 nc.vector.tensor_copy(out=ps_t, in_=eye)
                 for _ in range(ITER_NUM):
                     p2ps = nps.tile([P, P], f32, name="p2ps", tag="nsps")
                     nc.tensor.matmul(p2ps, ps_t, ps_t, skip_group_check=True)
                     p2s = nsp.tile([P, P], f32, name="p2s", tag="nsbig")
                     nc.scalar.activation(out=p2s, in_=p2ps, func=Act.Copy)
                     p3ps = nps.tile([P, P], f32, name="p3ps", tag="nsps")
                     nc.tensor.matmul(p3ps, p2s, ps_t, skip_group_check=True)
                     p3s = nsp.tile([P, P], f32, name="p3s", tag="nsbig")
                     nc.scalar.activation(out=p3s, in_=p3ps, func=Act.Copy)
                     tps = nps.tile([P, P], f32, name="tps", tag="nsps")
                     nc.tensor.matmul(tps, p3s, sign, skip_group_check=True)
                     ts = nsp.tile([P, P], f32, name="ts", tag="nsbig")
                     nc.scalar.activation(out=ts, in_=tps, func=Act.Identity,
                                          scale=-0.5)
                     pn = nsp.tile([P, P], f32, name=f"ps_{h}", tag="ps")
                     nc.vector.scalar_tensor_tensor(
                         out=pn, in0=ps_t, scalar=1.5, in1=ts,
                         op0=Alu.mult, op1=Alu.add)
                     ps_t = pn

                 # wm = P * rsqrt(trace)  (per-row group trace)
                 sq = nsp.tile([P, 1], f32, name="sq", tag="nssmall")
                 nc.scalar.activation(out=sq, in_=tvec, func=Act.Sqrt)
                 rs = nsp.tile([P, 1], f32, name="rs", tag="nssmall")
                 nc.vector.reciprocal(out=rs, in_=sq)
                 wmf = nsp.tile([P, P], f32, name="wmf", tag="nsbig")
                 nc.vector.tensor_scalar_mul(out=wmf, in0=ps_t, scalar1=rs)
                 nc.vector.tensor_copy(out=wm16[h], in_=wmf)

         # ================= PASS 2 =================
         with ExitStack() as ctx:
             stagep = ctx.enter_context(tc.tile_pool(name="stagep", bufs=2))
             yps = ctx.enter_context(tc.tile_pool(name="ypsum", bufs=4, space="PSUM"))

             for ci in range(NCHUNK if "nop2" not in variant else 0):
                 st = stagep.tile([P, UJ, C], f32, name="st")
                 for h in range(2):
                     # subtract mean in place (per-partition bias, fp16)
                     nc.scalar.activation(out=res[ci][h], in_=res[ci][h],
                                          func=Act.Identity, bias=nmu[h])
                     for b0 in range(0, UJ, 4):
                         bn = min(4, UJ - b0)
                         yp = yps.tile([P, 4, P], f32, name="yp")
                         for k in range(bn):
                             nc.tensor.matmul(yp[:, k, :],
                                              res[ci][h][:, b0 + k, :],
                                              wm16[h], skip_group_check=True)
                         nc.vector.tensor_copy(
                             out=st[:, b0:b0 + bn, h * P:(h + 1) * P],
                             in_=yp[:, :bn, :])
                 nc.gpsimd.dma_start(out=yv[ci], in_=st)

    nc.compile()
    return nc


def _get_nc(variant=()):
    key = ("nc",) + tuple(sorted(variant))
    if key not in _STATE:
        _STATE[key] = _build_nc(variant)
    return _STATE[key]


def _consts():
    g16 = np.eye(P, dtype=np.float16)
    eye = np.eye(P, dtype=np.float32)
    epseye = (EPS * np.eye(P)).astype(np.float32)
    mask = np.zeros((P, P), dtype=np.float32)
    for g in range(P // 16):
        mask[g * 16:(g + 1) * 16, g * 16:(g + 1) * 16] = 1.0
    return {"c_id16": g16, "c_eye": eye, "c_epseye": epseye, "c_mask": mask}


def _run(x, trace=False, variant=()):
    from concourse.bass_utils import run_bass_kernel_spmd

    x = np.ascontiguousarray(x, dtype=np.float32).reshape(B, W * H * C)
    consts = _consts()
    in_maps = []
    for i in range(N_CORES):
        m = {"x": np.ascontiguousarray(
            x[i * B_LOC:(i + 1) * B_LOC].reshape(N_LOC, C))}
        m.update(consts)
        in_maps.append(m)

    nc = _get_nc(variant)
    r = run_bass_kernel_spmd(nc, in_maps, core_ids=list(range(N_CORES)),
                             trace=trace)
    out = np.concatenate([r.results[i]["y"].reshape(B_LOC, W, H, C)
                          for i in range(N_CORES)], axis=0)
    return out, r


def kernel(inputs):
    return _run(inputs, trace=False)[0]


if __name__ == "__main__":
    x = np.random.randn(B, W, H, C).astype(np.float32)
    out, _ = _run(x)
    print(out.shape, out.dtype)



# revision 9
# speedup vs baseline: 1.2004x; 1.2004x over previous
"""Decorrelation (ZCA-whitening) normalization kernel for Trainium2 (Bass/Tile).

Full input (64, 56, 56, 256) f32. Data-parallel over batch across 8 NeuronCores
(8 batches -> 25088 pixels per core). Per core:

  Pass 1: stream pixel-major (128px, 14, 256ch) f32 chunks from HBM, cast to
          fp16 (DVE), accumulate per-half 128x128 second-moment Gram blocks on
          the PE (PSUM f32), PE-transpose every (128px,128ch) tile to
          channel-major fp16 (8-unit PSUM banks), drain on ACT with accum_out
          giving the per-channel sums for free. Resident fp16 tiles: 12.8 MB.
  Stats:  extract only the block-diagonal (group,16,16) Gram entries + channel
          sums -> one 18KB AllReduce. Newton-Schulz on f32r-bitcast matmuls,
          restructured as A=P^2, B=P*Sh (Sh=-0.5*sigma_n), P'=1.5P+A@B with the
          combine done by one DVE scalar_tensor_tensor reading PSUM directly.
          No separate mean-subtract pass: mean folds into pass 2.
  Pass 2: per PSUM tile, rank-1 init ones x (-wm@mu) (K=1 matmul), then the
          whitening matmuls accumulate on top; drains alternate DVE/ACT to
          staging, DMA out.

HBM traffic per core = 1x read + 1x write (pass 2 reads nothing from HBM).
"""

import sys

import numpy as np

for _p in ("/root/.axon_site/_ro/trn_rl_repo", "/opt/trn_rl_repo"):
    if _p not in sys.path:
        sys.path.append(_p)

# ---------------------------------------------------------------- constants
B, W, H, C = 64, 56, 56, 256
N_CORES = 8
B_LOC = B // N_CORES                # 8 batches per core
N_LOC = B_LOC * W * H               # 25088 pixels per core
N_TOT = B * W * H                   # 200704 pixels total
P = 128                             # partitions
UJ = 14                             # pixel-tiles (units) per chunk
CPX = UJ * P                        # 1792 pixels per chunk
NCHUNK = N_LOC // CPX               # 14 chunks per core
EPS = 1e-3
ITER_NUM = 5
NGRP = 8                            # 16x16 groups per 128-ch half
SB = 68                             # stats block: 2x32 Gd blocks + 2 sums + pad

assert NCHUNK * CPX == N_LOC

_STATE = {}


def _build_nc(variant=()):
    import concourse.bacc as bacc
    import concourse.tile as tile
    from concourse import mybir
    from contextlib import ExitStack

    f32 = mybir.dt.float32
    f32r = mybir.dt.float32r
    f16 = mybir.dt.float16
    Alu = mybir.AluOpType
    Act = mybir.ActivationFunctionType
    Axis = mybir.AxisListType

    nc = bacc.Bacc("TRN2", target_bir_lowering=False, debug=False,
                   num_devices=N_CORES)

    x = nc.dram_tensor("x", [N_LOC, C], f32, kind="ExternalInput").ap()
    y = nc.dram_tensor("y", [N_LOC, C], f32, kind="ExternalOutput").ap()
    c_id16 = nc.dram_tensor("c_id16", [P, P], f16, kind="ExternalInput").ap()
    c_eye = nc.dram_tensor("c_eye", [P, P], f32, kind="ExternalInput").ap()
    c_epseye = nc.dram_tensor("c_epseye", [P, P], f32, kind="ExternalInput").ap()
    c_mask = nc.dram_tensor("c_mask", [P, P], f32, kind="ExternalInput").ap()
    c_ones = nc.dram_tensor("c_ones", [1, P], f16, kind="ExternalInput").ap()

    ns_f32 = "nsf32" in variant

    with tile.TileContext(nc) as tc, ExitStack() as octx:
        # ---------------- long-lived pools
        consts = octx.enter_context(tc.tile_pool(name="consts", bufs=1))
        resp = octx.enter_context(tc.tile_pool(name="resident", bufs=1))
        statp = octx.enter_context(tc.tile_pool(name="stats", bufs=1))

        id16 = consts.tile([P, P], f16, name="id16")
        eye = consts.tile([P, P], f32, name="eye")
        epseye = consts.tile([P, P], f32, name="epseye")
        mask = consts.tile([P, P], f32, name="mask")
        ones16 = consts.tile([1, P], f16, name="ones16")
        nc.gpsimd.dma_start(out=id16, in_=c_id16)
        nc.gpsimd.dma_start(out=eye, in_=c_eye)
        nc.gpsimd.dma_start(out=epseye, in_=c_epseye)
        nc.gpsimd.dma_start(out=mask, in_=c_mask)
        nc.gpsimd.dma_start(out=ones16, in_=c_ones)

        # stats block for AllReduce: [Gd_a(16) | Gd_b(16) | s_a | s_b | pad]
        statsb = statp.tile([P, SB], f32, name="statsb")
        # per-drain accum_out columns: h0 -> cols 0..27, h1 -> cols 32..59
        acc_cols = statp.tile([P, 64], f32, name="acc_cols")

        # channel-major fp16 resident tiles: one per (chunk, half)
        res = [[resp.tile([P, UJ, P], f16, name=f"res_{c}_{h}")
                for h in range(2)] for c in range(NCHUNK)]

        xv = x.rearrange("(c j p) ch -> c p j ch", p=P, j=UJ)
        yv = y.rearrange("(c j p) ch -> c p j ch", p=P, j=UJ)

        # ================= PASS 1 =================
        with ExitStack() as ctx:
            loadp = ctx.enter_context(tc.tile_pool(name="loadp", bufs=3))
            castp = ctx.enter_context(tc.tile_pool(name="castp", bufs=2))
            gps = ctx.enter_context(tc.tile_pool(name="gpsum", bufs=1, space="PSUM"))
            trps = ctx.enter_context(tc.tile_pool(name="trpsum", bufs=4, space="PSUM"))

            g_ps = [gps.tile([P, P], f32, name=f"G_{h}") for h in range(2)]

            for ci in range(NCHUNK):
                xt = loadp.tile([P, UJ, C], f32, name="xt")
                nc.gpsimd.dma_start(out=xt, in_=xv[ci])
                xh = castp.tile([P, UJ, C], f16, name="xh")
                nc.vector.tensor_copy(out=xh, in_=xt)

                # Gram accumulation (fp16 in, f32 PSUM): G_h += T_h^T @ T_h
                for j in range(UJ):
                    first = ci == 0 and j == 0
                    last = ci == NCHUNK - 1 and j == UJ - 1
                    for h in range(2):
                        sl = xh[:, j, h * P:(h + 1) * P]
                        nc.tensor.matmul(g_ps[h], sl, sl, start=first,
                                         stop=last, skip_group_check=True)

                # PE transpose -> channel-major fp16; ACT drain with accum_out
                # (per-channel sums come for free from the drains)
                for h in range(2):
                    for t, b0 in enumerate(range(0, UJ, 8)):
                        bn = min(8, UJ - b0)
                        tp = trps.tile([P, 8, P], f16, name="tp")
                        for k in range(bn):
                            nc.tensor.matmul(
                                tp[:, k, :], xh[:, b0 + k, h * P:(h + 1) * P],
                                id16, is_transpose=True, skip_group_check=True)
                        col = h * 32 + 2 * ci + t
                        nc.scalar.activation(
                            out=res[ci][h][:, b0:b0 + bn, :], in_=tp[:, :bn, :],
                            func=Act.Copy,
                            accum_out=acc_cols[:, col:col + 1])

            # tail: 32-aligned block-diagonal Gram extract + channel sums
            # (each 32x32 block holds two 16x16 group blocks + junk that the
            # mask multiply kills later)
            for h in range(2):
                for k in range(4):
                    src = g_ps[h][32 * k:32 * (k + 1), 32 * k:32 * (k + 1)]
                    dst = statsb[32 * k:32 * (k + 1), h * 32:(h + 1) * 32]
                    if k % 2 == 0:
                        nc.scalar.activation(out=dst, in_=src, func=Act.Copy)
                    else:
                        nc.vector.tensor_copy(out=dst, in_=src)
            for h in range(2):
                nc.vector.reduce_sum(out=statsb[:, 64 + h:65 + h],
                                     in_=acc_cols[:, h * 32:h * 32 + 2 * NCHUNK],
                                     axis=Axis.X)
            nc.vector.memset(statsb[:, 66:SB], 0.0)

        # ================= ALL-REDUCE =================
        with ExitStack() as ctx:
            dramp = ctx.enter_context(tc.tile_pool(name="dram", bufs=1, space="DRAM"))
            cc_in = dramp.tile([P, SB], f32, name="cc_in")
            cc_out = dramp.tile([P, SB], f32, name="cc_out")
            arst = statp.tile([P, SB], f32, name="arst")
            if "nocc" in variant:
                nc.vector.tensor_scalar_mul(out=arst, in0=statsb,
                                            scalar1=float(N_CORES))
            else:
                nc.gpsimd.dma_start(out=cc_in, in_=statsb)
                nc.gpsimd.collective_compute(
                    "AllReduce", mybir.AluOpType.add,
                    replica_groups=[list(range(N_CORES))],
                    ins=[cc_in.opt()], outs=[cc_out.opt()])
                nc.gpsimd.dma_start(out=arst, in_=cc_out)

            # ============= Newton-Schulz (both halves interleaved) =========
            nsp = ctx.enter_context(tc.tile_pool(name="nsp", bufs=10))
            nps = ctx.enter_context(tc.tile_pool(name="nspsum", bufs=6, space="PSUM"))
            npsS = ctx.enter_context(tc.tile_pool(name="nspsumS", bufs=2, space="PSUM"))

            fns = f32 if ns_f32 else f32r

            wm16 = [statp.tile([P, P], f16, name=f"wm16_{h}") for h in range(2)]
            negwmu = [statp.tile([1, 4 * P], f16, name=f"negwmu_{h}")
                      for h in range(2)]

            # mu rows for both halves via one f32 PE transpose
            colpad = nsp.tile([P, P], f32, name="colpad", tag="nsbig")
            nc.vector.memset(colpad, 0.0)
            for h in range(2):
                nc.scalar.activation(out=colpad[:, 32 * h:32 * h + 1],
                                     in_=arst[:, 64 + h:65 + h],
                                     func=Act.Identity, scale=1.0 / N_TOT)
            rp_ps = nps.tile([P, P], f32, name="rp_ps", tag="nsps")
            nc.tensor.matmul(rp_ps, colpad, eye, is_transpose=True,
                             skip_group_check=True)
            rowpad = nsp.tile([P, P], f32, name="rowpad", tag="nsbig")
            nc.scalar.activation(out=rowpad, in_=rp_ps, func=Act.Copy)

            sig = []
            sh = []
            tvec = []
            for h in range(2):
                # sigma, scattered from the block-diagonal AllReduce payload
                # (scale (1-eps)/N folded into the scatter copies)
                sg = nsp.tile([P, P], f32, name=f"sig_{h}", tag="sig")
                nc.vector.memset(sg, 0.0)
                for k in range(4):
                    src = arst[32 * k:32 * (k + 1), h * 32:(h + 1) * 32]
                    dst = sg[32 * k:32 * (k + 1), 32 * k:32 * (k + 1)]
                    if k % 2 == 0:
                        nc.scalar.activation(out=dst, in_=src, func=Act.Identity,
                                             scale=(1.0 - EPS) / N_TOT)
                    else:
                        nc.vector.tensor_scalar_mul(out=dst, in0=src,
                                                    scalar1=(1.0 - EPS) / N_TOT)

                # outer product mu_h mu_h^T via K=1 rank-1 matmul
                o_ps = nps.tile([P, P], f32, name="o_ps", tag="nsps")
                nc.tensor.matmul(o_ps, rowpad[32 * h:32 * h + 1, :],
                                 rowpad[32 * h:32 * h + 1, :],
                                 skip_group_check=True)
                osc = nsp.tile([P, P], f32, name="osc", tag="nsbig")
                nc.scalar.activation(out=osc, in_=o_ps, func=Act.Identity,
                                     scale=-(1.0 - EPS))
                # sig = (sig + osc) * mask + eps*I
                nc.vector.tensor_add(out=sg, in0=sg, in1=osc)
                nc.vector.tensor_mul(out=sg, in0=sg, in1=mask)
                nc.vector.tensor_add(out=sg, in0=sg, in1=epseye)
                sig.append(sg)

                # per-group trace, spread back to rows via mask matmul
                djunk = nsp.tile([P, P], f32, name="djunk", tag="nsbig")
                dcol = nsp.tile([P, 1], f32, name="dcol", tag="nssmall")
                nc.vector.tensor_mul(out=djunk, in0=sg, in1=eye)
                nc.vector.reduce_sum(out=dcol, in_=djunk, axis=Axis.X)
                tv_ps = npsS.tile([P, 1], f32, name="tv_ps", tag="nsps1")
                nc.tensor.matmul(tv_ps, mask, dcol, skip_group_check=True)
                tv = nsp.tile([P, 1], f32, name=f"tvec_{h}", tag="nssmall")
                nc.scalar.activation(out=tv, in_=tv_ps, func=Act.Copy)
                tvec.append(tv)

                # Sh = -0.5 * sigma / trace
                rinv = nsp.tile([P, 1], f32, name="rinv", tag="nssmall")
                nc.vector.reciprocal(out=rinv, in_=tv)
                rneg = nsp.tile([P, 1], f32, name="rneg", tag="nssmall")
                nc.vector.tensor_scalar_mul(out=rneg, in0=rinv, scalar1=-0.5)
                s_t = nsp.tile([P, P], fns, name=f"sh_{h}", tag="sh")
                nc.vector.tensor_scalar_mul(out=s_t, in0=sg, scalar1=rneg)
                sh.append(s_t)

            # P1 = 1.5*I + Sh  (== iteration 1 with P0 = I)
            ps_t = []
            for h in range(2):
                p1 = nsp.tile([P, P], fns, name=f"ps_{h}", tag="ps")
                nc.vector.scalar_tensor_tensor(
                    out=p1, in0=eye, scalar=1.5, in1=sh[h],
                    op0=Alu.mult, op1=Alu.add)
                ps_t.append(p1)

            # remaining ITER_NUM-1 iterations, halves interleaved:
            #   A = P^2, B = P @ Sh, P' = 1.5*P + A @ B
            for it in range(ITER_NUM - 1):
                a_s = [None, None]
                b_s = [None, None]
                for h in range(2):
                    a_ps = nps.tile([P, P], f32, name="a_ps", tag="nsps")
                    nc.tensor.matmul(a_ps, ps_t[h], ps_t[h],
                                     skip_group_check=True)
                    b_ps = nps.tile([P, P], f32, name="b_ps", tag="nsps")
                    nc.tensor.matmul(b_ps, ps_t[h], sh[h],
                                     skip_group_check=True)
                    a_s[h] = nsp.tile([P, P], fns, name="a_s", tag="nsbig2")
                    nc.vector.tensor_copy(out=a_s[h], in_=a_ps)
                    b_s[h] = nsp.tile([P, P], fns, name="b_s", tag="nsbig2")
                    nc.scalar.activation(out=b_s[h], in_=b_ps, func=Act.Copy)
                for h in range(2):
                    c_ps = nps.tile([P, P], f32, name="c_ps", tag="nsps")
                    nc.tensor.matmul(c_ps, a_s[h], b_s[h],
                                     skip_group_check=True)
                    pn = nsp.tile([P, P], fns, name=f"ps_{h}", tag="ps")
                    nc.vector.scalar_tensor_tensor(
                        out=pn, in0=ps_t[h], scalar=1.5, in1=c_ps,
                        op0=Alu.mult, op1=Alu.add)
                    ps_t[h] = pn

            # wm = P * rsqrt(trace); negwmu = -(wm @ mu) as a tiled row
            for h in range(2):
                sq = nsp.tile([P, 1], f32, name="sq", tag="nssmall")
                nc.scalar.activation(out=sq, in_=tvec[h], func=Act.Sqrt)
                rs = nsp.tile([P, 1], f32, name="rs", tag="nssmall")
                nc.vector.reciprocal(out=rs, in_=sq)
                nc.vector.tensor_scalar_mul(out=wm16[h], in0=ps_t[h], scalar1=rs)

                mu16 = nsp.tile([P, 1], f16, name="mu16", tag="nssmall")
                nc.scalar.activation(out=mu16, in_=arst[:, 64 + h:65 + h],
                                     func=Act.Identity, scale=1.0 / N_TOT)
                w_ps = npsS.tile([1, P], f32, name="w_ps", tag="nsps1")
                nc.tensor.matmul(w_ps, mu16, wm16[h], skip_group_check=True)
                nc.scalar.activation(out=negwmu[h][:, 0:P], in_=w_ps,
                                     func=Act.Identity, scale=-1.0)
                for rep in range(1, 4):
                    nc.vector.tensor_copy(out=negwmu[h][:, rep * P:(rep + 1) * P],
                                          in_=negwmu[h][:, 0:P])

        # ================= PASS 2 =================
        with ExitStack() as ctx:
            stagep = ctx.enter_context(tc.tile_pool(name="stagep", bufs=2))
            yps = ctx.enter_context(tc.tile_pool(name="ypsum", bufs=6, space="PSUM"))

            for ci in range(NCHUNK):
                st = stagep.tile([P, UJ, C], f32, name="st")
                for h in range(2):
                    for t, b0 in enumerate(range(0, UJ, 4)):
                        bn = min(4, UJ - b0)
                        yp = yps.tile([P, 4, P], f32, name="yp")
                        # rank-1 init: yp[:, :bn, :] = ones x negwmu (mean fold)
                        nc.tensor.matmul(yp[:, :bn, :], ones16,
                                         negwmu[h][:, :bn * P],
                                         start=True, stop=False,
                                         skip_group_check=True)
                        for k in range(bn):
                            nc.tensor.matmul(yp[:, k, :],
                                             res[ci][h][:, b0 + k, :],
                                             wm16[h], start=False, stop=True,
                                             skip_group_check=True)
                        dst = st[:, b0:b0 + bn, h * P:(h + 1) * P]
                        if (t + 2 * h) % 2 == 0:
                            nc.vector.tensor_copy(out=dst, in_=yp[:, :bn, :])
                        else:
                            nc.scalar.activation(out=dst, in_=yp[:, :bn, :],
                                                 func=Act.Copy)
                nc.gpsimd.dma_start(out=yv[ci], in_=st)

    nc.compile()
    return nc


def _get_nc(variant=()):
    key = ("nc",) + tuple(sorted(variant))
    if key not in _STATE:
        _STATE[key] = _build_nc(variant)
    return _STATE[key]


def _consts():
    g16 = np.eye(P, dtype=np.float16)
    eye = np.eye(P, dtype=np.float32)
    epseye = (EPS * np.eye(P)).astype(np.float32)
    mask = np.zeros((P, P), dtype=np.float32)
    for g in range(P // 16):
        mask[g * 16:(g + 1) * 16, g * 16:(g + 1) * 16] = 1.0
    ones = np.ones((1, P), dtype=np.float16)
    return {"c_id16": g16, "c_eye": eye, "c_epseye": epseye, "c_mask": mask,
            "c_ones": ones}


def _run(x, trace=False, variant=()):
    from concourse.bass_utils import run_bass_kernel_spmd

    x = np.ascontiguousarray(x, dtype=np.float32).reshape(B, W * H * C)
    consts = _consts()
    in_maps = []
    for i in range(N_CORES):
        m = {"x": np.ascontiguousarray(
            x[i * B_LOC:(i + 1) * B_LOC].reshape(N_LOC, C))}
        m.update(consts)
        in_maps.append(m)

    nc = _get_nc(variant)
    r = run_bass_kernel_spmd(nc, in_maps, core_ids=list(range(N_CORES)),
                             trace=trace)
    out = np.concatenate([r.results[i]["y"].reshape(B_LOC, W, H, C)
                          for i in range(N_CORES)], axis=0)
    return out, r


def kernel(inputs):
    return _run(inputs, trace=False)[0]


if __name__ == "__main__":
    x = np.random.randn(B, W, H, C).astype(np.float32)
    out, _ = _run(x)
    print(out.shape, out.dtype)


# revision 10
# speedup vs baseline: 1.2905x; 1.0751x over previous
"""Decorrelation (ZCA-whitening) normalization kernel for Trainium2 (Bass/Tile).

Full input (64, 56, 56, 256) f32. Data-parallel over batch across 8 NeuronCores
(8 batches -> 25088 pixels per core). Per core:

  Pass 1: stream pixel-major (128px, 14, 256ch) f32 chunks from HBM, cast to
          fp16 (DVE), accumulate per-half 128x128 second-moment Gram blocks on
          the PE (PSUM f32), PE-transpose every (128px,128ch) tile to
          channel-major fp16 (8-unit PSUM banks), drain on ACT with accum_out
          giving the per-channel sums for free. Resident fp16 tiles: 12.8 MB.
  Stats:  extract only the block-diagonal (group,16,16) Gram entries + channel
          sums -> one 18KB AllReduce. Newton-Schulz on f32r-bitcast matmuls,
          restructured as A=P^2, B=P*Sh (Sh=-0.5*sigma_n), P'=1.5P+A@B with the
          combine done by one DVE scalar_tensor_tensor reading PSUM directly.
          No separate mean-subtract pass: mean folds into pass 2.
  Pass 2: per PSUM tile, rank-1 init ones x (-wm@mu) (K=1 matmul), then the
          whitening matmuls accumulate on top; drains alternate DVE/ACT to
          staging, DMA out.

HBM traffic per core = 1x read + 1x write (pass 2 reads nothing from HBM).
"""

import sys

import numpy as np

for _p in ("/root/.axon_site/_ro/trn_rl_repo", "/opt/trn_rl_repo"):
    if _p not in sys.path:
        sys.path.append(_p)

# ---------------------------------------------------------------- constants
B, W, H, C = 64, 56, 56, 256
N_CORES = 8
B_LOC = B // N_CORES                # 8 batches per core
N_LOC = B_LOC * W * H               # 25088 pixels per core
N_TOT = B * W * H                   # 200704 pixels total
P = 128                             # partitions
UJ = 14                             # pixel-tiles (units) per chunk
CPX = UJ * P                        # 1792 pixels per chunk
NCHUNK = N_LOC // CPX               # 14 chunks per core
EPS = 1e-3
ITER_NUM = 5
NGRP = 8                            # 16x16 groups per 128-ch half
SB = 68                             # stats block: 2x32 Gd blocks + 2 sums + pad

assert NCHUNK * CPX == N_LOC

_STATE = {}


def _build_nc(variant=()):
    import concourse.bacc as bacc
    import concourse.tile as tile
    from concourse import mybir
    from contextlib import ExitStack

    f32 = mybir.dt.float32
    f32r = mybir.dt.float32r
    f16 = mybir.dt.float16
    Alu = mybir.AluOpType
    Act = mybir.ActivationFunctionType
    Axis = mybir.AxisListType

    nc = bacc.Bacc("TRN2", target_bir_lowering=False, debug=False,
                   num_devices=N_CORES)

    x = nc.dram_tensor("x", [N_LOC, C], f32, kind="ExternalInput").ap()
    y = nc.dram_tensor("y", [N_LOC, C], f32, kind="ExternalOutput").ap()
    c_id16 = nc.dram_tensor("c_id16", [P, P], f16, kind="ExternalInput").ap()
    c_eye = nc.dram_tensor("c_eye", [P, P], f32, kind="ExternalInput").ap()
    c_epseye = nc.dram_tensor("c_epseye", [P, P], f32, kind="ExternalInput").ap()
    c_mask = nc.dram_tensor("c_mask", [P, P], f32, kind="ExternalInput").ap()

    ns_f32 = "nsf32" in variant

    with tile.TileContext(nc) as tc, ExitStack() as octx:
        # ---------------- long-lived pools
        consts = octx.enter_context(tc.tile_pool(name="consts", bufs=1))
        resp = octx.enter_context(tc.tile_pool(name="resident", bufs=1))
        statp = octx.enter_context(tc.tile_pool(name="stats", bufs=1))

        id16 = consts.tile([P, P], f16, name="id16")
        eye = consts.tile([P, P], f32, name="eye")
        epseye = consts.tile([P, P], f32, name="epseye")
        mask = consts.tile([P, P], f32, name="mask")

        # stats block for AllReduce: [Gd_a(16) | Gd_b(16) | s_a | s_b | pad]
        statsb = statp.tile([P, SB], f32, name="statsb")
        # per-drain accum_out columns: h0 -> cols 0..27, h1 -> cols 32..59
        acc_cols = statp.tile([P, 64], f32, name="acc_cols")

        # channel-major fp16 resident tiles: one per (chunk, half)
        res = [[resp.tile([P, UJ, P], f16, name=f"res_{c}_{h}")
                for h in range(2)] for c in range(NCHUNK)]

        xv = x.rearrange("(c j p) ch -> c p j ch", p=P, j=UJ)
        yv = y.rearrange("(c j p) ch -> c p j ch", p=P, j=UJ)

        # ================= PASS 1 =================
        with ExitStack() as ctx:
            loadp = ctx.enter_context(tc.tile_pool(name="loadp", bufs=3))
            castp = ctx.enter_context(tc.tile_pool(name="castp", bufs=2))
            gps = ctx.enter_context(tc.tile_pool(name="gpsum", bufs=1, space="PSUM"))
            trps = ctx.enter_context(tc.tile_pool(name="trpsum", bufs=4, space="PSUM"))

            g_ps = [gps.tile([P, P], f32, name=f"G_{h}") for h in range(2)]

            for ci in range(NCHUNK):
                xt = loadp.tile([P, UJ, C], f32, name="xt")
                nc.gpsimd.dma_start(out=xt, in_=xv[ci])
                if ci == 0:
                    # consts ride behind the first chunk load on the DGE queue
                    nc.gpsimd.dma_start(out=id16, in_=c_id16)
                    nc.gpsimd.dma_start(out=eye, in_=c_eye)
                    nc.gpsimd.dma_start(out=epseye, in_=c_epseye)
                    nc.gpsimd.dma_start(out=mask, in_=c_mask)
                xh = castp.tile([P, UJ, C], f16, name="xh")
                nc.vector.tensor_copy(out=xh, in_=xt)

                # Gram accumulation (fp16 in, f32 PSUM): G_h += T_h^T @ T_h
                for j in range(UJ):
                    first = ci == 0 and j == 0
                    last = ci == NCHUNK - 1 and j == UJ - 1
                    for h in range(2):
                        sl = xh[:, j, h * P:(h + 1) * P]
                        nc.tensor.matmul(g_ps[h], sl, sl, start=first,
                                         stop=last, skip_group_check=True)

                # PE transpose -> channel-major fp16; ACT drain with accum_out
                # (per-channel sums come for free from the drains)
                for h in range(2):
                    for t, b0 in enumerate(range(0, UJ, 8)):
                        bn = min(8, UJ - b0)
                        tp = trps.tile([P, 8, P], f16, name="tp")
                        for k in range(bn):
                            nc.tensor.matmul(
                                tp[:, k, :], xh[:, b0 + k, h * P:(h + 1) * P],
                                id16, is_transpose=True, skip_group_check=True)
                        col = h * 32 + 2 * ci + t
                        nc.scalar.activation(
                            out=res[ci][h][:, b0:b0 + bn, :], in_=tp[:, :bn, :],
                            func=Act.Copy,
                            accum_out=acc_cols[:, col:col + 1])

            # tail: 32-aligned block-diagonal Gram extract + channel sums
            # (each 32x32 block holds two 16x16 group blocks + junk that the
            # mask multiply kills later)
            for h in range(2):
                for k in range(4):
                    src = g_ps[h][32 * k:32 * (k + 1), 32 * k:32 * (k + 1)]
                    dst = statsb[32 * k:32 * (k + 1), h * 32:(h + 1) * 32]
                    if k % 2 == 0:
                        nc.scalar.activation(out=dst, in_=src, func=Act.Copy)
                    else:
                        nc.vector.tensor_copy(out=dst, in_=src)
            for h in range(2):
                nc.vector.reduce_sum(out=statsb[:, 64 + h:65 + h],
                                     in_=acc_cols[:, h * 32:h * 32 + 2 * NCHUNK],
                                     axis=Axis.X)
            nc.vector.memset(statsb[:, 66:SB], 0.0)

        # ================= ALL-REDUCE =================
        with ExitStack() as ctx:
            dramp = ctx.enter_context(tc.tile_pool(name="dram", bufs=1, space="DRAM"))
            cc_in = dramp.tile([P, SB], f32, name="cc_in")
            cc_out = dramp.tile([P, SB], f32, name="cc_out")
            arst = statp.tile([P, SB], f32, name="arst")
            if "nocc" in variant:
                nc.vector.tensor_scalar_mul(out=arst, in0=statsb,
                                            scalar1=float(N_CORES))
            else:
                nc.gpsimd.dma_start(out=cc_in, in_=statsb)
                nc.gpsimd.collective_compute(
                    "AllReduce", mybir.AluOpType.add,
                    replica_groups=[list(range(N_CORES))],
                    ins=[cc_in.opt()], outs=[cc_out.opt()])
                nc.gpsimd.dma_start(out=arst, in_=cc_out)

            # ============= Newton-Schulz (both halves interleaved) =========
            nsp = ctx.enter_context(tc.tile_pool(name="nsp", bufs=10))
            nps = ctx.enter_context(tc.tile_pool(name="nspsum", bufs=6, space="PSUM"))
            npsS = ctx.enter_context(tc.tile_pool(name="nspsumS", bufs=2, space="PSUM"))

            fns = f32 if ns_f32 else f32r

            wm16 = [statp.tile([P, P], f16, name=f"wm16_{h}") for h in range(2)]
            nmu = [statp.tile([P, 1], f32, name=f"nmu_{h}") for h in range(2)]

            # -mean columns first (DVE), so the ACT bias-subtract pass can
            # start while Newton-Schulz runs (NS itself never touches ACT)
            for h in range(2):
                nc.vector.tensor_scalar_mul(out=nmu[h],
                                            in0=arst[:, 64 + h:65 + h],
                                            scalar1=-1.0 / N_TOT)

            # mu rows for both halves via one f32 PE transpose
            colpad = nsp.tile([P, P], f32, name="colpad", tag="nsbig")
            nc.vector.memset(colpad, 0.0)
            for h in range(2):
                nc.vector.tensor_scalar_mul(out=colpad[:, 32 * h:32 * h + 1],
                                            in0=arst[:, 64 + h:65 + h],
                                            scalar1=1.0 / N_TOT)
            rp_ps = nps.tile([P, P], f32, name="rp_ps", tag="nsps")
            nc.tensor.matmul(rp_ps, colpad, eye, is_transpose=True,
                             skip_group_check=True)
            rowpad = nsp.tile([P, P], f32, name="rowpad", tag="nsbig")
            nc.vector.tensor_copy(out=rowpad, in_=rp_ps)

            sig = []
            sh = []
            tvec = []
            for h in range(2):
                # sigma, scattered from the block-diagonal AllReduce payload
                # (scale (1-eps)/N folded into the scatter copies)
                sg = nsp.tile([P, P], f32, name=f"sig_{h}", tag="sig")
                nc.vector.memset(sg, 0.0)
                for k in range(4):
                    src = arst[32 * k:32 * (k + 1), h * 32:(h + 1) * 32]
                    dst = sg[32 * k:32 * (k + 1), 32 * k:32 * (k + 1)]
                    nc.vector.tensor_scalar_mul(out=dst, in0=src,
                                                scalar1=(1.0 - EPS) / N_TOT)

                # outer product mu_h mu_h^T via K=1 rank-1 matmul
                o_ps = nps.tile([P, P], f32, name="o_ps", tag="nsps")
                nc.tensor.matmul(o_ps, rowpad[32 * h:32 * h + 1, :],
                                 rowpad[32 * h:32 * h + 1, :],
                                 skip_group_check=True)
                osc = nsp.tile([P, P], f32, name="osc", tag="nsbig")
                nc.vector.tensor_scalar_mul(out=osc, in0=o_ps,
                                            scalar1=-(1.0 - EPS))
                # sig = (sig + osc) * mask + eps*I
                nc.vector.tensor_add(out=sg, in0=sg, in1=osc)
                nc.vector.tensor_mul(out=sg, in0=sg, in1=mask)
                nc.vector.tensor_add(out=sg, in0=sg, in1=epseye)
                sig.append(sg)

                # per-group trace, spread back to rows via mask matmul
                djunk = nsp.tile([P, P], f32, name="djunk", tag="nsbig")
                dcol = nsp.tile([P, 1], f32, name="dcol", tag="nssmall")
                nc.vector.tensor_mul(out=djunk, in0=sg, in1=eye)
                nc.vector.reduce_sum(out=dcol, in_=djunk, axis=Axis.X)
                tv_ps = npsS.tile([P, 1], f32, name="tv_ps", tag="nsps1")
                nc.tensor.matmul(tv_ps, mask, dcol, skip_group_check=True)

                # Sh = -0.5 * sigma / trace;  rs = rsqrt(trace) for later
                rinv = nsp.tile([P, 1], f32, name=f"rinv_{h}", tag="nssmall")
                nc.vector.reciprocal(out=rinv, in_=tv_ps)
                rs = nsp.tile([P, 1], f32, name=f"rs_{h}", tag="nssmall")
                nc.scalar.activation(out=rs, in_=rinv, func=Act.Sqrt)
                tvec.append(rs)
                rneg = nsp.tile([P, 1], f32, name="rneg", tag="nssmall")
                nc.vector.tensor_scalar_mul(out=rneg, in0=rinv, scalar1=-0.5)
                s_t = nsp.tile([P, P], fns, name=f"sh_{h}", tag="sh")
                nc.vector.tensor_scalar_mul(out=s_t, in0=sg, scalar1=rneg)
                sh.append(s_t)

            # P1 = 1.5*I + Sh  (== iteration 1 with P0 = I)
            ps_t = []
            for h in range(2):
                p1 = nsp.tile([P, P], fns, name=f"ps_{h}", tag="ps")
                nc.vector.scalar_tensor_tensor(
                    out=p1, in0=eye, scalar=1.5, in1=sh[h],
                    op0=Alu.mult, op1=Alu.add)
                ps_t.append(p1)

            # remaining ITER_NUM-1 iterations, halves interleaved:
            #   A = P^2, B = P @ Sh, P' = 1.5*P + A @ B
            for it in range(ITER_NUM - 1):
                a_s = [None, None]
                b_s = [None, None]
                for h in range(2):
                    a_ps = nps.tile([P, P], f32, name="a_ps", tag="nsps")
                    nc.tensor.matmul(a_ps, ps_t[h], ps_t[h],
                                     skip_group_check=True)
                    b_ps = nps.tile([P, P], f32, name="b_ps", tag="nsps")
                    nc.tensor.matmul(b_ps, ps_t[h], sh[h],
                                     skip_group_check=True)
                    a_s[h] = nsp.tile([P, P], fns, name="a_s", tag="nsbig2")
                    nc.vector.tensor_copy(out=a_s[h], in_=a_ps)
                    b_s[h] = nsp.tile([P, P], fns, name="b_s", tag="nsbig2")
                    nc.vector.tensor_copy(out=b_s[h], in_=b_ps)
                for h in range(2):
                    c_ps = nps.tile([P, P], f32, name="c_ps", tag="nsps")
                    nc.tensor.matmul(c_ps, a_s[h], b_s[h],
                                     skip_group_check=True)
                    pn = nsp.tile([P, P], fns, name=f"ps_{h}", tag="ps")
                    nc.vector.scalar_tensor_tensor(
                        out=pn, in0=ps_t[h], scalar=1.5, in1=c_ps,
                        op0=Alu.mult, op1=Alu.add)
                    ps_t[h] = pn

            # wm = P * rsqrt(trace)
            for h in range(2):
                nc.vector.tensor_scalar_mul(out=wm16[h], in0=ps_t[h],
                                            scalar1=tvec[h])

            # mean-subtract the resident tiles in place on ACT, overlapping
            # with the (ACT-free) Newton-Schulz above
            for ci in range(NCHUNK):
                for h in range(2):
                    nc.scalar.activation(out=res[ci][h], in_=res[ci][h],
                                         func=Act.Identity, bias=nmu[h])

        # ================= PASS 2 =================
        with ExitStack() as ctx:
            stagep = ctx.enter_context(tc.tile_pool(name="stagep", bufs=2))
            yps = ctx.enter_context(tc.tile_pool(name="ypsum", bufs=6, space="PSUM"))

            for ci in range(NCHUNK):
                st = stagep.tile([P, UJ, C], f32, name="st")
                for h in range(2):
                    for t, b0 in enumerate(range(0, UJ, 4)):
                        bn = min(4, UJ - b0)
                        yp = yps.tile([P, 4, P], f32, name="yp")
                        for k in range(bn):
                            nc.tensor.matmul(yp[:, k, :],
                                             res[ci][h][:, b0 + k, :],
                                             wm16[h], skip_group_check=True)
                        dst = st[:, b0:b0 + bn, h * P:(h + 1) * P]
                        # ACT is busy with the bias pass for the first chunks
                        if ci < 8 or (t + h) % 2 == 0:
                            nc.vector.tensor_copy(out=dst, in_=yp[:, :bn, :])
                        else:
                            nc.scalar.activation(out=dst, in_=yp[:, :bn, :],
                                                 func=Act.Copy)
                nc.gpsimd.dma_start(out=yv[ci], in_=st)

    nc.compile()
    return nc


def _get_nc(variant=()):
    key = ("nc",) + tuple(sorted(variant))
    if key not in _STATE:
        _STATE[key] = _build_nc(variant)
    return _STATE[key]


def _consts():
    g16 = np.eye(P, dtype=np.float16)
    eye = np.eye(P, dtype=np.float32)
    epseye = (EPS * np.eye(P)).astype(np.float32)
    mask = np.zeros((P, P), dtype=np.float32)
    for g in range(P // 16):
        mask[g * 16:(g + 1) * 16, g * 16:(g + 1) * 16] = 1.0
    return {"c_id16": g16, "c_eye": eye, "c_epseye": epseye, "c_mask": mask}


def _run(x, trace=False, variant=()):
    from concourse.bass_utils import run_bass_kernel_spmd

    x = np.ascontiguousarray(x, dtype=np.float32).reshape(B, W * H * C)
    consts = _consts()
    in_maps = []
    for i in range(N_CORES):
        m = {"x": np.ascontiguousarray(
            x[i * B_LOC:(i + 1) * B_LOC].reshape(N_LOC, C))}
        m.update(consts)
        in_maps.append(m)

    nc = _get_nc(variant)
    r = run_bass_kernel_spmd(nc, in_maps, core_ids=list(range(N_CORES)),
                             trace=trace)
    out = np.concatenate([r.results[i]["y"].reshape(B_LOC, W, H, C)
                          for i in range(N_CORES)], axis=0)
    return out, r


def kernel(inputs):
    return _run(inputs, trace=False)[0]


if __name__ == "__main__":
    x = np.random.randn(B, W, H, C).astype(np.float32)
    out, _ = _run(x)
    print(out.shape, out.dtype)


# revision 15
# speedup vs baseline: 1.3423x; 1.0401x over previous
"""Decorrelation (ZCA-whitening) normalization kernel for Trainium2 (Bass/Tile).

Full input (64, 56, 56, 256) f32. Data-parallel over batch across 8 NeuronCores
(8 batches -> 25088 pixels per core). Per core:

  Pass 1: stream pixel-major (128px, 14, 256ch) f32 chunks from HBM, cast to
          fp16 (DVE), accumulate per-half 128x128 second-moment Gram blocks on
          the PE (PSUM f32), PE-transpose every (128px,128ch) tile to
          channel-major fp16 (8-unit PSUM banks), drain on ACT with accum_out
          giving the per-channel sums for free. Resident fp16 tiles: 12.8 MB.
  Stats:  extract only the block-diagonal (group,16,16) Gram entries + channel
          sums -> one 18KB AllReduce. Newton-Schulz on f32r-bitcast matmuls,
          restructured as A=P^2, B=P*Sh (Sh=-0.5*sigma_n), P'=1.5P+A@B with the
          combine done by one DVE scalar_tensor_tensor reading PSUM directly.
          No separate mean-subtract pass: mean folds into pass 2.
  Pass 2: per PSUM tile, rank-1 init ones x (-wm@mu) (K=1 matmul), then the
          whitening matmuls accumulate on top; drains alternate DVE/ACT to
          staging, DMA out.

HBM traffic per core = 1x read + 1x write (pass 2 reads nothing from HBM).
"""

import sys

import numpy as np

for _p in ("/root/.axon_site/_ro/trn_rl_repo", "/opt/trn_rl_repo"):
    if _p not in sys.path:
        sys.path.append(_p)

# ---------------------------------------------------------------- constants
B, W, H, C = 64, 56, 56, 256
N_CORES = 8
B_LOC = B // N_CORES                # 8 batches per core
N_LOC = B_LOC * W * H               # 25088 pixels per core
N_TOT = B * W * H                   # 200704 pixels total
P = 128                             # partitions
UJ = 14                             # pixel-tiles (units) per chunk
CPX = UJ * P                        # 1792 pixels per chunk
NCHUNK = N_LOC // CPX               # 14 chunks per core
EPS = 1e-3
ITER_NUM = 5
NGRP = 8                            # 16x16 groups per 128-ch half
SB = 68                             # stats block: 2x32 Gd blocks + 2 sums + pad

assert NCHUNK * CPX == N_LOC

_STATE = {}


def _build_nc(variant=()):
    import concourse.bacc as bacc
    import concourse.tile as tile
    from concourse import mybir
    from contextlib import ExitStack

    f32 = mybir.dt.float32
    f32r = mybir.dt.float32r
    f16 = mybir.dt.float16
    Alu = mybir.AluOpType
    Act = mybir.ActivationFunctionType
    Axis = mybir.AxisListType

    nc = bacc.Bacc("TRN2", target_bir_lowering=False, debug=False,
                   num_devices=N_CORES)

    x = nc.dram_tensor("x", [N_LOC, C], f32, kind="ExternalInput").ap()
    y = nc.dram_tensor("y", [N_LOC, C], f32, kind="ExternalOutput").ap()
    c_id16 = nc.dram_tensor("c_id16", [P, P], f16, kind="ExternalInput").ap()
    c_eye = nc.dram_tensor("c_eye", [P, P], f32, kind="ExternalInput").ap()
    c_epseye = nc.dram_tensor("c_epseye", [P, P], f32, kind="ExternalInput").ap()
    c_mask = nc.dram_tensor("c_mask", [P, P], f32, kind="ExternalInput").ap()

    ns_f32 = "nsf32" in variant

    with tile.TileContext(nc) as tc, ExitStack() as octx:
        # ---------------- long-lived pools
        consts = octx.enter_context(tc.tile_pool(name="consts", bufs=1))
        resp = octx.enter_context(tc.tile_pool(name="resident", bufs=1))
        statp = octx.enter_context(tc.tile_pool(name="stats", bufs=1))

        id16 = consts.tile([P, P], f16, name="id16")
        eye = consts.tile([P, P], f32, name="eye")
        epseye = consts.tile([P, P], f32, name="epseye")
        mask = consts.tile([P, P], f32, name="mask")

        # stats block for AllReduce: [Gd_a(32) | Gd_b(32) | s_a | s_b | pad]
        statsb = statp.tile([P, SB], f32, name="statsb")
        use_p2p = "p2p" in variant
        if use_p2p:
            # manual all-reduce: every core XOR-broadcasts its stats into slot
            # k of peer (me^k)'s gather buffer; slot k thus holds core me^k.
            gbuf = statp.tile([P, 8, SB], f32, name="gbuf")
            red4 = statp.tile([P, 4, SB], f32, name="red4")
            red2 = statp.tile([P, 2, SB], f32, name="red2")
            rsem = nc.alloc_semaphore("p2p_arrive")
            lsem = nc.alloc_semaphore("p2p_sent")
        # per-drain accum_out columns: h0 -> cols 0..27, h1 -> cols 32..59
        acc_cols = statp.tile([P, 64], f32, name="acc_cols")

        # channel-major fp16 resident tiles: one per (chunk, half)
        res = [[resp.tile([P, UJ, P], f16, name=f"res_{c}_{h}")
                for h in range(2)] for c in range(NCHUNK)]

        xv = x.rearrange("(c j p) ch -> c p j ch", p=P, j=UJ)
        yv = y.rearrange("(c j p) ch -> c p j ch", p=P, j=UJ)

        # ================= PASS 1 =================
        with ExitStack() as ctx:
            loadp = ctx.enter_context(tc.tile_pool(name="loadp", bufs=3))
            castp = ctx.enter_context(tc.tile_pool(name="castp", bufs=2))
            gps = ctx.enter_context(tc.tile_pool(name="gpsum", bufs=1, space="PSUM"))
            trps = ctx.enter_context(tc.tile_pool(name="trpsum", bufs=4, space="PSUM"))

            g_ps = [gps.tile([P, P], f32, name=f"G_{h}") for h in range(2)]

            for ci in range(NCHUNK):
                xt = loadp.tile([P, UJ, C], f32, name="xt")
                nc.gpsimd.dma_start(out=xt, in_=xv[ci])
                if ci == 0:
                    # consts ride behind the first chunk load on the DGE queue
                    nc.gpsimd.dma_start(out=id16, in_=c_id16)
                    nc.gpsimd.dma_start(out=eye, in_=c_eye)
                    nc.gpsimd.dma_start(out=epseye, in_=c_epseye)
                    nc.gpsimd.dma_start(out=mask, in_=c_mask)
                if ci == 1 and use_p2p:
                    # descriptor generation early; source read deferred until
                    # trigger_dma fires after statsb is complete
                    for k in range(1, 8):
                        rd = [None] * 8
                        rd[k] = (0, k)
                        nc.gpsimd.remote_dma_broadcast(
                            out_ap=gbuf[:, k, :], in_ap=statsb,
                            remote_sem=rsem, local_sem=lsem, rdests=rd)
                xh = castp.tile([P, UJ, C], f16, name="xh")
                nc.vector.tensor_copy(out=xh, in_=xt)

                # Gram accumulation (fp16 in, f32 PSUM): G_h += T_h^T @ T_h
                for j in range(UJ):
                    first = ci == 0 and j == 0
                    last = ci == NCHUNK - 1 and j == UJ - 1
                    for h in range(2):
                        sl = xh[:, j, h * P:(h + 1) * P]
                        nc.tensor.matmul(g_ps[h], sl, sl, start=first,
                                         stop=last, skip_group_check=True)

                # PE transpose -> channel-major fp16; ACT drain with accum_out
                # (per-channel sums come for free from the drains)
                for h in range(2):
                    for t, b0 in enumerate(range(0, UJ, 8)):
                        bn = min(8, UJ - b0)
                        tp = trps.tile([P, 8, P], f16, name="tp")
                        for k in range(bn):
                            nc.tensor.matmul(
                                tp[:, k, :], xh[:, b0 + k, h * P:(h + 1) * P],
                                id16, is_transpose=True, skip_group_check=True)
                        col = h * 32 + 2 * ci + t
                        nc.scalar.activation(
                            out=res[ci][h][:, b0:b0 + bn, :], in_=tp[:, :bn, :],
                            func=Act.Copy,
                            accum_out=acc_cols[:, col:col + 1])

            # tail: 32-aligned block-diagonal Gram extract + channel sums
            # (each 32x32 block holds two 16x16 group blocks + junk that the
            # mask multiply kills later)
            for h in range(2):
                for k in range(4):
                    src = g_ps[h][32 * k:32 * (k + 1), 32 * k:32 * (k + 1)]
                    dst = statsb[32 * k:32 * (k + 1), h * 32:(h + 1) * 32]
                    if k % 2 == 0:
                        nc.scalar.activation(out=dst, in_=src, func=Act.Copy)
                    else:
                        nc.vector.tensor_copy(out=dst, in_=src)
            for h in range(2):
                nc.vector.reduce_sum(out=statsb[:, 64 + h:65 + h],
                                     in_=acc_cols[:, h * 32:h * 32 + 2 * NCHUNK],
                                     axis=Axis.X)
            nc.vector.memset(statsb[:, 66:SB], 0.0)

        # ================= ALL-REDUCE =================
        with ExitStack() as ctx:
            arst = statp.tile([P, SB], f32, name="arst")
            if use_p2p:
                nc.vector.tensor_copy(out=gbuf[:, 0, :], in_=statsb)
                nc.gpsimd.trigger_dma(count=None)
                # the scheduling sim cannot see cross-core sem increments, so
                # the arrival wait (p2p_arrive >= 14: 7 peers x 2 engines) is
                # attached to this instruction after tile scheduling
                p2p_gate_inst = nc.vector.tensor_tensor(
                    out=red4, in0=gbuf[:, 0:4, :], in1=gbuf[:, 4:8, :],
                    op=Alu.add)
                nc.vector.tensor_tensor(out=red2, in0=red4[:, 0:2, :],
                                        in1=red4[:, 2:4, :], op=Alu.add)
                nc.vector.tensor_tensor(out=arst, in0=red2[:, 0, :],
                                        in1=red2[:, 1, :], op=Alu.add)
            else:
                dramp = ctx.enter_context(
                    tc.tile_pool(name="dram", bufs=1, space="DRAM"))
                cc_in = dramp.tile([P, SB], f32, name="cc_in")
                cc_out = dramp.tile([P, SB], f32, name="cc_out")
                nc.gpsimd.dma_start(out=cc_in, in_=statsb)
                nc.gpsimd.collective_compute(
                    "AllReduce", mybir.AluOpType.add,
                    replica_groups=[list(range(N_CORES))],
                    ins=[cc_in.opt()], outs=[cc_out.opt()])
                nc.gpsimd.dma_start(out=arst, in_=cc_out)

            # ============= Newton-Schulz (both halves interleaved) =========
            nsp = ctx.enter_context(tc.tile_pool(name="nsp", bufs=10))
            nps = ctx.enter_context(tc.tile_pool(name="nspsum", bufs=6, space="PSUM"))
            npsS = ctx.enter_context(tc.tile_pool(name="nspsumS", bufs=2, space="PSUM"))

            fns = f32 if ns_f32 else f32r

            wm16 = [statp.tile([P, P], f16, name=f"wm16_{h}") for h in range(2)]
            nmu = [statp.tile([P, 1], f32, name=f"nmu_{h}") for h in range(2)]

            # -mean columns first (DVE), so the ACT bias-subtract pass can
            # start while Newton-Schulz runs (NS itself never touches ACT)
            for h in range(2):
                nc.vector.tensor_scalar_mul(out=nmu[h],
                                            in0=arst[:, 64 + h:65 + h],
                                            scalar1=-1.0 / N_TOT)

            # mu rows for both halves via one f32 PE transpose
            colpad = nsp.tile([P, P], f32, name="colpad", tag="nsbig")
            nc.vector.memset(colpad, 0.0)
            for h in range(2):
                nc.vector.tensor_scalar_mul(out=colpad[:, 32 * h:32 * h + 1],
                                            in0=arst[:, 64 + h:65 + h],
                                            scalar1=1.0 / N_TOT)
            rp_ps = nps.tile([P, P], f32, name="rp_ps", tag="nsps")
            nc.tensor.matmul(rp_ps, colpad, eye, is_transpose=True,
                             skip_group_check=True)
            rowpad = nsp.tile([P, P], f32, name="rowpad", tag="nsbig")
            nc.vector.tensor_copy(out=rowpad, in_=rp_ps)

            sig = []
            sh = []
            tvec = []
            for h in range(2):
                # sigma, scattered from the block-diagonal AllReduce payload
                # (scale (1-eps)/N folded into the scatter copies)
                sg = nsp.tile([P, P], f32, name=f"sig_{h}", tag="sig")
                nc.vector.memset(sg, 0.0)
                for k in range(4):
                    src = arst[32 * k:32 * (k + 1), h * 32:(h + 1) * 32]
                    dst = sg[32 * k:32 * (k + 1), 32 * k:32 * (k + 1)]
                    nc.vector.tensor_scalar_mul(out=dst, in0=src,
                                                scalar1=(1.0 - EPS) / N_TOT)

                # outer product mu_h mu_h^T via K=1 rank-1 matmul
                o_ps = nps.tile([P, P], f32, name="o_ps", tag="nsps")
                nc.tensor.matmul(o_ps, rowpad[32 * h:32 * h + 1, :],
                                 rowpad[32 * h:32 * h + 1, :],
                                 skip_group_check=True)
                osc = nsp.tile([P, P], f32, name="osc", tag="nsbig")
                nc.vector.tensor_scalar_mul(out=osc, in0=o_ps,
                                            scalar1=-(1.0 - EPS))
                # sig = (sig + osc) * mask + eps*I
                nc.vector.tensor_add(out=sg, in0=sg, in1=osc)
                nc.vector.tensor_mul(out=sg, in0=sg, in1=mask)
                nc.vector.tensor_add(out=sg, in0=sg, in1=epseye)
                sig.append(sg)

                # per-group trace, spread back to rows via mask matmul
                djunk = nsp.tile([P, P], f32, name="djunk", tag="nsbig")
                dcol = nsp.tile([P, 1], f32, name="dcol", tag="nssmall")
                nc.vector.tensor_mul(out=djunk, in0=sg, in1=eye)
                nc.vector.reduce_sum(out=dcol, in_=djunk, axis=Axis.X)
                tv_ps = npsS.tile([P, 1], f32, name="tv_ps", tag="nsps1")
                nc.tensor.matmul(tv_ps, mask, dcol, skip_group_check=True)

                # Sh = -0.5 * sigma / trace;  rs = rsqrt(trace) for later
                rinv = nsp.tile([P, 1], f32, name=f"rinv_{h}", tag="nssmall")
                nc.vector.reciprocal(out=rinv, in_=tv_ps)
                rs = nsp.tile([P, 1], f32, name=f"rs_{h}", tag="nssmall")
                nc.scalar.activation(out=rs, in_=rinv, func=Act.Sqrt)
                tvec.append(rs)
                rneg = nsp.tile([P, 1], f32, name="rneg", tag="nssmall")
                nc.vector.tensor_scalar_mul(out=rneg, in0=rinv, scalar1=-0.5)
                s_t = nsp.tile([P, P], fns, name=f"sh_{h}", tag="sh")
                nc.vector.tensor_scalar_mul(out=s_t, in0=sg, scalar1=rneg)
                sh.append(s_t)

            # P1 = 1.5*I + Sh  (== iteration 1 with P0 = I)
            ps_t = []
            for h in range(2):
                p1 = nsp.tile([P, P], fns, name=f"ps_{h}", tag="ps")
                nc.vector.scalar_tensor_tensor(
                    out=p1, in0=eye, scalar=1.5, in1=sh[h],
                    op0=Alu.mult, op1=Alu.add)
                ps_t.append(p1)

            # remaining ITER_NUM-1 iterations, halves interleaved:
            #   A = P^2, B = P @ Sh, P' = 1.5*P + A @ B
            for it in range(ITER_NUM - 1):
                a_s = [None, None]
                b_s = [None, None]
                for h in range(2):
                    a_ps = nps.tile([P, P], f32, name="a_ps", tag="nsps")
                    nc.tensor.matmul(a_ps, ps_t[h], ps_t[h],
                                     skip_group_check=True)
                    b_ps = nps.tile([P, P], f32, name="b_ps", tag="nsps")
                    nc.tensor.matmul(b_ps, ps_t[h], sh[h],
                                     skip_group_check=True)
                    a_s[h] = nsp.tile([P, P], fns, name="a_s", tag="nsbig2")
                    nc.vector.tensor_copy(out=a_s[h], in_=a_ps)
                    b_s[h] = nsp.tile([P, P], fns, name="b_s", tag="nsbig2")
                    nc.vector.tensor_copy(out=b_s[h], in_=b_ps)
                for h in range(2):
                    c_ps = nps.tile([P, P], f32, name="c_ps", tag="nsps")
                    nc.tensor.matmul(c_ps, a_s[h], b_s[h],
                                     skip_group_check=True)
                    pn = nsp.tile([P, P], fns, name=f"ps_{h}", tag="ps")
                    nc.vector.scalar_tensor_tensor(
                        out=pn, in0=ps_t[h], scalar=1.5, in1=c_ps,
                        op0=Alu.mult, op1=Alu.add)
                    ps_t[h] = pn

            # wm = P * rsqrt(trace)
            for h in range(2):
                nc.vector.tensor_scalar_mul(out=wm16[h], in0=ps_t[h],
                                            scalar1=tvec[h])

            # mean-subtract the resident tiles in place on ACT, overlapping
            # with the (ACT-free) Newton-Schulz above
            for ci in range(NCHUNK):
                for h in range(2):
                    nc.scalar.activation(out=res[ci][h], in_=res[ci][h],
                                         func=Act.Identity, bias=nmu[h])

        # ================= PASS 2 =================
        with ExitStack() as ctx:
            stagep = ctx.enter_context(tc.tile_pool(name="stagep", bufs=3))
            yps = ctx.enter_context(tc.tile_pool(name="ypsum", bufs=6, space="PSUM"))

            for ci in range(NCHUNK):
                st = stagep.tile([P, UJ, C], f32, name="st")
                # block order (t, h) so each half-chunk [j0:j0+8) completes
                # early and its DMA can start while the rest drains
                for t, b0 in enumerate(range(0, UJ, 4)):
                    bn = min(4, UJ - b0)
                    for h in range(2):
                        yp = yps.tile([P, 4, P], f32, name="yp")
                        for k in range(bn):
                            nc.tensor.matmul(yp[:, k, :],
                                             res[ci][h][:, b0 + k, :],
                                             wm16[h], skip_group_check=True)
                        dst = st[:, b0:b0 + bn, h * P:(h + 1) * P]
                        # ACT is busy with the bias pass for the first chunks
                        if ci < 8 or (t + h) % 2 == 0:
                            nc.vector.tensor_copy(out=dst, in_=yp[:, :bn, :])
                        else:
                            nc.scalar.activation(out=dst, in_=yp[:, :bn, :],
                                                 func=Act.Copy)
                    if t == 1:
                        nc.gpsimd.dma_start(out=yv[ci][:, 0:8, :],
                                            in_=st[:, 0:8, :])
                nc.gpsimd.dma_start(out=yv[ci][:, 8:UJ, :],
                                    in_=st[:, 8:UJ, :])

    if use_p2p:
        w = mybir.SyncWait(sync_type="semaphore", id=rsem.num,
                           wait_mode="sem-ge-imm", wait_value=14,
                           ant_name="p2p_arrive")
        inst = p2p_gate_inst.ins
        si = inst.sync_info
        if si is None:
            inst.sync_info = mybir.SyncInfo(on_wait=[w], on_update=[])
        else:
            inst.sync_info = mybir.SyncInfo(on_wait=list(si.on_wait) + [w],
                                            on_update=list(si.on_update))
    nc.compile()
    return nc


def _get_nc(variant=()):
    key = ("nc",) + tuple(sorted(variant))
    if key not in _STATE:
        _STATE[key] = _build_nc(variant)
    return _STATE[key]


def _consts():
    g16 = np.eye(P, dtype=np.float16)
    eye = np.eye(P, dtype=np.float32)
    epseye = (EPS * np.eye(P)).astype(np.float32)
    mask = np.zeros((P, P), dtype=np.float32)
    for g in range(P // 16):
        mask[g * 16:(g + 1) * 16, g * 16:(g + 1) * 16] = 1.0
    return {"c_id16": g16, "c_eye": eye, "c_epseye": epseye, "c_mask": mask}


def _run(x, trace=False, variant=()):
    from concourse.bass_utils import run_bass_kernel_spmd

    x = np.ascontiguousarray(x, dtype=np.float32).reshape(B, W * H * C)
    consts = _consts()
    in_maps = []
    for i in range(N_CORES):
        m = {"x": np.ascontiguousarray(
            x[i * B_LOC:(i + 1) * B_LOC].reshape(N_LOC, C))}
        m.update(consts)
        in_maps.append(m)

    nc = _get_nc(variant)
    r = run_bass_kernel_spmd(nc, in_maps, core_ids=list(range(N_CORES)),
                             trace=trace)
    out = np.concatenate([r.results[i]["y"].reshape(B_LOC, W, H, C)
                          for i in range(N_CORES)], axis=0)
    return out, r


def kernel(inputs):
    return _run(inputs, trace=False)[0]


if __name__ == "__main__":
    x = np.random.randn(B, W, H, C).astype(np.float32)
    out, _ = _run(x)
    print(out.shape, out.dtype)
